# revision 7
# baseline (speedup 1.0000x reference)
# Trainium2 Bass kernel for LlamaAttention (HF RoPE, GQA, causal) —
# tensor-parallel over heads across 8 NeuronCores.
#
# Sharding: each core owns 4 query heads + 1 KV head (Wq/Wk/Wv sharded on the
# output/head dim, Wo on the input dim). x is replicated (pre-transposed +
# bf16-cast on host); each core produces a partial o_proj output [B*S, D] in
# fp32 which the host all-reduces (sums) and reshapes.
#
# Device program (identical on all cores, per-core data via in_maps):
#   phase 1: QKV projection (bf16 matmul, fp32 PSUM accum), RoPE on q/k in
#            [tok, hd] layout (free-dim shifts), DMA-transpose q/k -> qT/kT.
#   phase 2: per (batch, head): scores = qT.T @ kT per 128-row q-block
#            (causal: only lower blocks), additive mask on diagonal block,
#            exp on ScalarE with fused row-sum (accum_out), normalize P on
#            VectorE, DMA-transpose P tiles, PV matmuls accumulate out^T.
#   phase 3: o_proj: y[tok, :] += aT.T @ Wo_shard, DMA straight PSUM->DRAM.

import math
import os
import sys

for _p in ("/opt/trn_rl_repo", "/root/.axon_site/_ro/trn_rl_repo"):
    if os.path.isdir(_p) and _p not in sys.path:
        sys.path.insert(0, _p)

import numpy as np
import ml_dtypes

import concourse.bass as bass
import concourse.mybir as mybir
import concourse.tile as tile
from concourse.bass_utils import run_bass_kernel_spmd

BF16 = ml_dtypes.bfloat16
MDT_BF = mybir.dt.bfloat16
MDT_F32 = mybir.dt.float32

N_CORES = 8


class KCfg:
    """Shapes in units of 128. Defaults = the real problem (per-core view)."""

    def __init__(self, n_dd=32, n_pos=16, batch=2, hq=4, dout=4096):
        self.n_dd = n_dd          # contraction tiles: D = 128 * n_dd
        self.n_pos = n_pos        # seq blocks:        S = 128 * n_pos
        self.batch = batch
        self.hq = hq              # local query heads (head_dim = 128)
        self.dout = dout          # o_proj output features
        self.S = 128 * n_pos
        self.D = 128 * n_dd
        self.TB = batch * n_pos   # token blocks
        self.qw = 128 * hq
        self.ddw = self.qw + 256  # per-d-tile packed width of [Wq | Wk | Wv]

    def ptoff(self, j):
        # packed column offset of PT tile row j (tiles (j, i) for i >= j)
        return 128 * (self.n_pos * j - j * (j - 1) // 2)

    @property
    def ptw(self):
        return self.ptoff(self.n_pos)


FULL = KCfg()


def emit(tc, io, cfg: KCfg):
    """Emit the single-core program. io: dict of DRAM APs."""
    nc = tc.nc
    n_dd, n_pos, B, HQ, DOUT = cfg.n_dd, cfg.n_pos, cfg.batch, cfg.hq, cfg.dout
    S, TB, qw, ddw = cfg.S, cfg.TB, cfg.qw, cfg.ddw
    scale = 1.0 / math.sqrt(128.0)
    xt_d, wqkv_d, wo_d = io["xt"], io["wqkv"], io["wo"]
    cos_d, ssin_d, mask_d, y_d = io["cos"], io["ssin"], io["mask"], io["y"]

    with (
        tc.tile_pool(name="static", bufs=1) as stp,
        tc.tile_pool(name="psA", bufs=2, space="PSUM") as psA,
        tc.tile_pool(name="psB", bufs=2, space="PSUM") as psB,
        tc.tile_pool(name="psO", bufs=2, space="PSUM") as psO,
    ):
        qT_all = stp.tile([128, B * HQ * S], MDT_BF, tag="qT")
        kT_all = stp.tile([128, B * S], MDT_BF, tag="kT")
        v_all = stp.tile([128, B * S], MDT_BF, tag="vv")
        mask_sb = stp.tile([128, 128], MDT_F32, tag="mask")
        nc.sync.dma_start(out=mask_sb[:], in_=mask_d[:])

        # ---------------- phase 1: QKV projection + RoPE ----------------
        with (
            tc.tile_pool(name="p1c", bufs=1) as p1c,
            tc.tile_pool(name="p1s", bufs=3) as p1s,
            tc.tile_pool(name="p1r", bufs=4) as p1r,
        ):
            wqkv_sb = p1c.tile([128, n_dd * ddw], MDT_BF, tag="wqkv")
            cos_sb = p1c.tile([128, TB * 128], MDT_F32, tag="cos")
            ssin_sb = p1c.tile([128, TB * 128], MDT_F32, tag="ssin")
            nc.sync.dma_start(out=wqkv_sb[:], in_=wqkv_d[:])
            nc.sync.dma_start(out=cos_sb[:], in_=cos_d[:])
            nc.sync.dma_start(out=ssin_sb[:], in_=ssin_d[:])

            for t in range(TB):
                b, pos = divmod(t, n_pos)
                xt_sb = p1s.tile([128, n_dd * 128], MDT_BF, tag="xt")
                nc.sync.dma_start(out=xt_sb[:], in_=xt_d[t])
                psq = psA.tile([128, 512], MDT_F32, tag="a")
                pskv = psB.tile([128, 256], MDT_F32, tag="b")
                for dd in range(n_dd):
                    st, sp = dd == 0, dd == n_dd - 1
                    lhsT = xt_sb[:, dd * 128:(dd + 1) * 128]
                    nc.tensor.matmul(
                        psq[:, :qw], lhsT,
                        wqkv_sb[:, dd * ddw: dd * ddw + qw],
                        start=st, stop=sp,
                    )
                    nc.tensor.matmul(
                        pskv[:], lhsT,
                        wqkv_sb[:, dd * ddw + qw: (dd + 1) * ddw],
                        start=st, stop=sp,
                    )
                cs = cos_sb[:, t * 128:(t + 1) * 128]
                sn = ssin_sb[:, t * 128:(t + 1) * 128]
                for hc in range(HQ + 1):  # HQ query chunks then 1 key chunk
                    src = (
                        psq[:, hc * 128:(hc + 1) * 128]
                        if hc < HQ
                        else pskv[:, 0:128]
                    )
                    t1 = p1r.tile([128, 128], MDT_F32, tag="t1")
                    t2 = p1r.tile([128, 128], MDT_F32, tag="t2")
                    rr = p1r.tile([128, 128], MDT_BF, tag="rr")
                    nc.vector.tensor_mul(t1[:], src, cs)
                    nc.vector.tensor_mul(t2[:, 0:64], src[:, 64:128], sn[:, 0:64])
                    nc.vector.tensor_mul(t2[:, 64:128], src[:, 0:64], sn[:, 64:128])
                    nc.vector.tensor_add(rr[:], t1[:], t2[:])
                    if hc < HQ:
                        dst = qT_all[
                            :, ((b * HQ + hc) * n_pos + pos) * 128:
                               ((b * HQ + hc) * n_pos + pos + 1) * 128
                        ]
                    else:
                        dst = kT_all[
                            :, (b * n_pos + pos) * 128:(b * n_pos + pos + 1) * 128
                        ]
                    nc.sync.dma_start_transpose(out=dst, in_=rr[:])
                nc.scalar.copy(
                    out=v_all[:, (b * n_pos + pos) * 128:(b * n_pos + pos + 1) * 128],
                    in_=pskv[:, 128:256],
                )

        # ---------------- phase 2 + 3: attention + o_proj ----------------
        with (
            tc.tile_pool(name="p2c", bufs=1) as p2c,
            tc.tile_pool(name="p2p", bufs=6) as p2p,
            tc.tile_pool(name="p2s", bufs=4) as p2s,
            tc.tile_pool(name="paT", bufs=2) as paT,
            tc.tile_pool(name="p2y", bufs=3) as p2y,
        ):
            wo_sb = p2c.tile([128, HQ * DOUT], MDT_BF, tag="wo")
            nc.sync.dma_start(out=wo_sb[:], in_=wo_d[:])
            PT_sb = p2c.tile([128, cfg.ptw], MDT_BF, tag="PT")

            for b in range(B):
                aT_b = paT.tile([128, HQ * S], MDT_BF, tag="aT")
                for h in range(HQ):
                    qoff = (b * HQ + h) * S
                    for i in range(n_pos):
                        fd = (i + 1) * 128
                        nch = (fd + 511) // 512
                        acc = p2s.tile([128, 4], MDT_F32, tag="acc")
                        pchs = []
                        for ci in range(nch):
                            c0 = ci * 512
                            w = min(512, fd - c0)
                            ps_s = psA.tile([128, 512], MDT_F32, tag="a")
                            nc.tensor.matmul(
                                ps_s[:, :w],
                                qT_all[:, qoff + i * 128: qoff + (i + 1) * 128],
                                kT_all[:, b * S + c0: b * S + c0 + w],
                                start=True, stop=True,
                            )
                            if c0 <= i * 128 < c0 + w:
                                dc = i * 128 - c0
                                nc.vector.tensor_add(
                                    ps_s[:, dc:dc + 128],
                                    ps_s[:, dc:dc + 128],
                                    mask_sb[:],
                                )
                            pch = p2p.tile([128, 512], MDT_BF, tag="p")
                            nc.scalar.activation(
                                out=pch[:, :w], in_=ps_s[:, :w],
                                func=mybir.ActivationFunctionType.Exp,
                                scale=float(scale),
                                accum_out=acc[:, ci:ci + 1],
                            )
                            pchs.append((pch, c0, w))
                        if nch > 1:
                            rs = p2s.tile([128, 1], MDT_F32, tag="rs")
                            nc.vector.tensor_reduce(
                                out=rs[:], in_=acc[:, :nch],
                                axis=mybir.AxisListType.X, op=mybir.AluOpType.add,
                            )
                            rs_ap = rs[:]
                        else:
                            rs_ap = acc[:, 0:1]
                        rc = p2s.tile([128, 1], MDT_F32, tag="rc")
                        nc.vector.reciprocal(rc[:], rs_ap)
                        for (pch, c0, w) in pchs:
                            pn = p2p.tile([128, 512], MDT_BF, tag="pn")
                            nc.vector.tensor_scalar_mul(pn[:, :w], pch[:, :w], rc[:])
                            for m in range(0, w, 128):
                                j = (c0 + m) // 128
                                nc.sync.dma_start_transpose(
                                    out=PT_sb[
                                        :, cfg.ptoff(j) + (i - j) * 128:
                                           cfg.ptoff(j) + (i - j + 1) * 128
                                    ],
                                    in_=pn[:, m:m + 128],
                                )
                        po = psB.tile([128, 128], MDT_F32, tag="b")
                        for j in range(i + 1):
                            nc.tensor.matmul(
                                po[:],
                                v_all[:, (b * n_pos + j) * 128:(b * n_pos + j + 1) * 128],
                                PT_sb[
                                    :, cfg.ptoff(j) + (i - j) * 128:
                                       cfg.ptoff(j) + (i - j + 1) * 128
                                ],
                                start=(j == 0), stop=(j == i),
                            )
                        nc.scalar.copy(
                            out=aT_b[:, h * S + i * 128: h * S + (i + 1) * 128],
                            in_=po[:],
                        )
                # o_proj for this batch
                for t in range(n_pos):
                    row0 = (b * n_pos + t) * 128
                    for c0 in range(0, DOUT, 1024):
                        cw = min(1024, DOUT - c0)
                        py = psO.tile([128, 1024], MDT_F32, tag="o")
                        for fo in range(HQ):
                            st, sp = fo == 0, fo == HQ - 1
                            for s0 in range(0, cw, 512):
                                sw = min(512, cw - s0)
                                nc.tensor.matmul(
                                    py[:, s0:s0 + sw],
                                    aT_b[:, fo * S + t * 128: fo * S + (t + 1) * 128],
                                    wo_sb[:, fo * DOUT + c0 + s0: fo * DOUT + c0 + s0 + sw],
                                    start=st, stop=sp,
                                )
                        ysb = p2y.tile([128, 1024], MDT_F32, tag="y")
                        if (t + c0 // 1024) % 2 == 0:
                            nc.scalar.copy(out=ysb[:, :cw], in_=py[:, :cw])
                        else:
                            nc.vector.tensor_copy(ysb[:, :cw], py[:, :cw])
                        nc.sync.dma_start(
                            out=y_d[row0:row0 + 128, c0:c0 + cw], in_=ysb[:, :cw]
                        )


# ---------------------------------------------------------------------------
# host side
# ---------------------------------------------------------------------------

def _spill_excess_waits(nc, limit=1):
    """walrus codegen allows very few inline sem-waits on several opcodes
    (TensorTensor/DMACopy/DmaTransposeAnt = 1). Spill excess on_wait entries
    into standalone EventSemaphore wait instructions on the same engine,
    immediately before the instruction (same semantics: the sequencer blocks
    on them in program order)."""
    n = 0
    for bb in nc.main_func.blocks:
        out = []
        for ins in bb.instructions:
            si = getattr(ins, "sync_info", None)
            waits = list(si.on_wait) if si is not None and si.on_wait else []
            if len(waits) > limit and type(ins).__name__ != "InstEventSemaphore":
                keep = waits[-limit:]
                for w in waits[:-limit]:
                    ev = mybir.InstEventSemaphore(
                        name=f"{ins.name}_w{n}",
                        engine=ins.engine,
                        ins=[],
                        outs=[],
                        sync_info=mybir.SyncInfo(on_wait=[w], on_update=[]),
                    )
                    out.append(ev)
                    n += 1
                si.on_wait = keep
            out.append(ins)
        bb.instructions[:] = out
    return n


def build_nc(cfg: KCfg = FULL, spill=True):
    nc = bass.Bass()
    io = {
        "xt": nc.declare_dram_parameter(
            "xt", [cfg.TB, 128, cfg.n_dd * 128], MDT_BF, isOutput=False
        ),
        "wqkv": nc.declare_dram_parameter(
            "wqkv", [128, cfg.n_dd * cfg.ddw], MDT_BF, isOutput=False
        ),
        "wo": nc.declare_dram_parameter(
            "wo", [128, cfg.hq * cfg.dout], MDT_BF, isOutput=False
        ),
        "cos": nc.declare_dram_parameter(
            "cos", [128, cfg.TB * 128], MDT_F32, isOutput=False
        ),
        "ssin": nc.declare_dram_parameter(
            "ssin", [128, cfg.TB * 128], MDT_F32, isOutput=False
        ),
        "mask": nc.declare_dram_parameter("mask", [128, 128], MDT_F32, isOutput=False),
        "y": nc.declare_dram_parameter(
            "y", [cfg.TB * 128, cfg.dout], MDT_F32, isOutput=True
        ),
    }
    io = {k: (v.ap() if hasattr(v, "ap") else v) for k, v in io.items()}
    with tile.TileContext(nc) as tc:
        emit(tc, io, cfg)
    if spill:
        _spill_excess_waits(nc)
    return nc


def rope_tables(S, hd=128, theta=10000.0):
    inv_freq = 1.0 / (theta ** (np.arange(0, hd, 2, dtype=np.float32) / hd))
    pos = np.arange(S, dtype=np.float32)
    freqs = pos[:, None] * inv_freq[None, :]
    emb = np.concatenate([freqs, freqs], axis=1)
    cos = np.cos(emb).astype(np.float32)
    sin = np.sin(emb).astype(np.float32)
    ssin = sin.copy()
    ssin[:, : hd // 2] = -sin[:, : hd // 2]
    return cos, ssin


def host_inputs(x, Wq, Wk, Wv, Wo, cfg: KCfg = FULL, n_cores=N_CORES):
    """Build per-core in_maps. Shards: core c owns q heads [c*hq,(c+1)*hq),
    kv head c."""
    B, S, D = x.shape
    assert (B, S, D) == (cfg.batch, cfg.S, cfg.D)
    T = B * S
    x_flat = np.asarray(x, np.float32).reshape(T, D)
    xt = np.ascontiguousarray(
        x_flat.T.astype(BF16)
        .reshape(cfg.n_dd, 128, cfg.TB, 128)
        .transpose(2, 1, 0, 3)
        .reshape(cfg.TB, 128, cfg.n_dd * 128)
    )
    cos, ssin = rope_tables(S)
    cos_full = np.concatenate([cos] * B, axis=0)
    ssin_full = np.concatenate([ssin] * B, axis=0)
    cos_pk = np.ascontiguousarray(
        cos_full.reshape(cfg.TB, 128, 128).transpose(1, 0, 2).reshape(128, cfg.TB * 128)
    )
    ssin_pk = np.ascontiguousarray(
        ssin_full.reshape(cfg.TB, 128, 128).transpose(1, 0, 2).reshape(128, cfg.TB * 128)
    )
    mask = np.where(np.tril(np.ones((128, 128), bool)), 0.0, -1e9).astype(np.float32)

    qw, kvw = cfg.qw, 128
    in_maps = []
    for c in range(n_cores):
        wq_s = np.asarray(Wq[:, c * qw:(c + 1) * qw], np.float32).astype(BF16)
        wk_s = np.asarray(Wk[:, c * kvw:(c + 1) * kvw], np.float32).astype(BF16)
        wv_s = np.asarray(Wv[:, c * kvw:(c + 1) * kvw], np.float32).astype(BF16)
        wqkv = np.concatenate(
            [
                np.concatenate(
                    [
                        wq_s.reshape(cfg.n_dd, 128, qw)[dd],
                        wk_s.reshape(cfg.n_dd, 128, kvw)[dd],
                        wv_s.reshape(cfg.n_dd, 128, kvw)[dd],
                    ],
                    axis=1,
                )
                for dd in range(cfg.n_dd)
            ],
            axis=1,
        )
        wo_s = (
            np.asarray(Wo[c * qw:(c + 1) * qw], np.float32)
            .astype(BF16)
            .reshape(cfg.hq, 128, cfg.dout)
        )
        wo_pk = np.concatenate([wo_s[fo] for fo in range(cfg.hq)], axis=1)
        in_maps.append(
            {
                "xt": xt,
                "wqkv": np.ascontiguousarray(wqkv),
                "wo": np.ascontiguousarray(wo_pk),
                "cos": cos_pk,
                "ssin": ssin_pk,
                "mask": mask,
            }
        )
    return in_maps


_NC_CACHE = {}


def kernel(x, Wq, Wk, Wv, Wo):
    cfg = FULL
    if "nc" not in _NC_CACHE:
        _NC_CACHE["nc"] = build_nc(cfg)
    nc = _NC_CACHE["nc"]
    in_maps = host_inputs(x, Wq, Wk, Wv, Wo, cfg)
    res = run_bass_kernel_spmd(nc, in_maps, list(range(N_CORES)))
    y = np.zeros((cfg.TB * 128, cfg.dout), np.float32)
    for r in res.results:
        y += np.asarray(r["y"], np.float32)
    return y.reshape(cfg.batch, cfg.S, cfg.dout).astype(np.float32)


if __name__ == "__main__":
    # smoke: build the full program and print instruction counts
    nc = build_nc()
    print("built OK")
